# revision 2
# baseline (speedup 1.0000x reference)
"""Trainium2 Bass kernel for MixtralBlockSparseTop2MLP grouped-GEMM MoE.

Problem: 4096 rows (sorted by expert), 8 experts, hidden=1024, ffn=3584.
  out[r] = silu(x[r] @ W1g[e(r)]) * (x[r] @ W1u[e(r)]) @ W2[e(r)]

Sharding: tensor-parallel over the ffn dimension. Each of the 8 cores gets
a 448-channel slice of every expert's gate/up/down weights and computes a
partial output for ALL 4096 rows; the host sums the 8 partials. All cores
run the identical program (segment structure baked from rows_for_experts at
call time), so one SPMD NEFF serves all 8 cores with per-core weight data.

v2 design notes (vs the 203us baseline):
  - gemm2 is flipped: stationary = w2 [ffn_k x 128 H-cols], moving = the
    a-tiles [ffn_k x nch rows], psum = [128 H-cols x nch]. Cost scales with
    actual chunk rows instead of ceil(rows/128) full 512-col slices, which
    removes the partial-slice and zero-pad PE waste entirely. Output is
    H-major ([P, KO2, rows]); the host transposes while summing partials.
  - the 4th gemm2 k-tile runs as a 64-partition matmul (448 = 3*128 + 64),
    so no zero padding of a-tiles or w2 anywhere (less DMA, no memsets).
  - chunks are balanced per segment (n split into ceil(n/512) near-equal
    parts) so no chunk sits at the ~100ns/instruction issue floor with only
    8-15 rows of work.
  - x and out use per-chunk packed DRAM layouts (one contiguous line per
    partition per transfer) - no padded-row DMA waste, minimal descriptors.
  - DMA rings: gpsimd(SWDGE)=x loads only, sync(HWDGE)=w1 + output stores,
    scalar(HWDGE)=w2. psum->sbuf casts alternate vector/scalar.

Compute dtype: bf16 matmul inputs with fp32 PSUM accumulation (fp32 matmul
is 4x slower; fp8 fails the 2e-2 gate - measured 3.8-6.5% rel err).
"""

import sys

sys.path.insert(0, "/opt/trn_rl_repo")

import numpy as np
import ml_dtypes

E, R, H, F = 8, 1024 * 4, 1024, 3584
FC = F // 8          # 448 ffn channels per core
P = 128
KO = H // P          # 8 k-tiles for gemm1
K2 = (FC + P - 1) // P   # 4 k-tiles for gemm2 (last has 64 rows)
KO2 = H // P         # 8 output m-tiles for gemm2
NCH = 512            # max row-chunk (PSUM bank free dim)

BF16 = ml_dtypes.bfloat16

# test.py introspection: last BassKernelResults from run_bass_kernel_spmd
LAST_RESULT = None

_PROGRAM_CACHE = {}


def _segments(rows_for_experts):
    """[(expert, row_start, n_rows)] for experts with n_rows > 0."""
    segs = []
    r0 = 0
    for e in range(E):
        n = int(rows_for_experts[e])
        if n > 0:
            segs.append((e, r0, n))
        r0 += n
    # largest segment first (amortizes the startup weight-load stall),
    # smallest last (short end-of-kernel gemm2+store chain).
    segs.sort(key=lambda s: -s[2])
    return segs


def _chunk_sizes(n):
    """Split n rows into ceil(n/NCH) near-equal chunks (balanced so no
    chunk is tiny enough to be instruction-issue bound)."""
    k = (n + NCH - 1) // NCH
    base, rem = divmod(n, k)
    return [base + 1] * rem + [base] * (k - rem)


def _chunk_list(segments):
    """[(expert, row_start, nch)] in program iteration order."""
    out = []
    for (e, r0, n_e) in segments:
        c0 = 0
        for nch in _chunk_sizes(n_e):
            out.append((e, r0 + c0, nch))
            c0 += nch
    return out


def _build_program(segments, act_mode="silu"):
    import concourse.mybir as mybir
    import concourse.tile as tile
    from concourse import bacc

    dt = mybir.dt
    nc = bacc.Bacc(None, target_bir_lowering=False, debug=False)

    chunks = _chunk_list(segments)
    n_chunks = len(chunks)
    # per-chunk packed x: chunk c occupies columns [KO*r : KO*(r+nch)] as a
    # row-major [KO, nch] block per partition (one contiguous line each).
    xTp = nc.declare_dram_parameter("xTp", [P, KO * R], dt.bfloat16, isOutput=False)
    w1 = nc.declare_dram_parameter("w1c", [E, P, KO, 2 * FC], dt.bfloat16, isOutput=False)
    w2m = nc.declare_dram_parameter("w2m", [E, P, 3, H], dt.bfloat16, isOutput=False)
    w2t = nc.declare_dram_parameter("w2t", [E, 64, H], dt.bfloat16, isOutput=False)
    # per-chunk packed out: chunk c occupies columns [KO2*r : KO2*(r+nch)]
    # as a [KO2, nch] block per partition; out[r, m*128+p] = outp[p, ...].
    outp = nc.declare_dram_parameter("outp", [P, KO2 * R], dt.bfloat16, isOutput=True)

    silu = mybir.ActivationFunctionType.Silu
    sigmoid = mybir.ActivationFunctionType.Sigmoid
    copyf = mybir.ActivationFunctionType.Copy

    with tile.TileContext(nc) as tc:
        with (
            tc.tile_pool(name="w1p", bufs=4) as w1p,
            tc.tile_pool(name="w2p", bufs=3) as w2p,
            tc.tile_pool(name="xp", bufs=4) as xp,
            tc.tile_pool(name="apool", bufs=2) as apool,
            tc.tile_pool(name="spool", bufs=2) as spool,
            tc.tile_pool(name="opool", bufs=3) as opool,
            tc.tile_pool(name="hps", bufs=5, space="PSUM") as hps,
            tc.tile_pool(name="ops", bufs=3, space="PSUM") as ops,
        ):
            chunk_idx = 0
            first = True
            pending_gemm2 = None
            for (e, r0, n_e) in segments:
                w1sb = w1p.tile([P, KO, 2 * FC], dt.bfloat16, tag="w1sb")
                if first:
                    # prologue: interleave x-chunk0 / w1 k-slices so both
                    # k0 pieces are first on their rings; split w1 k0 so
                    # the very first matmul's 128 columns land fastest.
                    nch0 = _chunk_sizes(n_e)[0]
                    xsb0 = xp.tile([P, KO, NCH], dt.bfloat16, tag="xsb")
                    for k in range(KO):
                        off = KO * r0 + k * nch0
                        nc.gpsimd.dma_start(
                            xsb0[:, k, :nch0], xTp[:, off : off + nch0]
                        )
                        if k == 0:
                            nc.sync.dma_start(w1sb[:, 0, 0:P], w1[e, :, 0, 0:P])
                            nc.sync.dma_start(
                                w1sb[:, 0, P:], w1[e, :, 0, P:]
                            )
                        else:
                            nc.sync.dma_start(w1sb[:, k, :], w1[e, :, k, :])
                    first = False
                else:
                    xsb0 = None
                    nc.sync.dma_start(w1sb[:], w1[e])
                w2sb = w2p.tile([P, K2, H], dt.bfloat16, tag="w2sb")
                nc.scalar.dma_start(w2sb[:, 0:3, :], w2m[e])
                nc.scalar.dma_start(w2sb[0:64, 3, :], w2t[e])

                c0 = 0
                for nch in _chunk_sizes(n_e):
                    r = r0 + c0
                    c0 += nch

                    if xsb0 is not None:
                        xsb, xsb0 = xsb0, None
                    else:
                        xsb = xp.tile([P, KO, NCH], dt.bfloat16, tag="xsb")
                        off = KO * r
                        nc.gpsimd.dma_start(
                            xsb[:, :, :nch], xTp[:, off : off + KO * nch]
                        )

                    # gemm1: 7 packed m-slices [gate_u(64) | up_u(64)];
                    # psum_u partitions 0:64 = gate, 64:128 = up.
                    # silu via ACT into a 64-row tmp, then DVE cross-base
                    # multiply into the packed a k-tiles. a3 rows 64:128
                    # are never written or read (448 = 3.5 k-tiles).
                    a_tiles = [
                        apool.tile([P, NCH], dt.bfloat16, tag=f"a{j}", name=f"a{j}")
                        for j in range(K2)
                    ]
                    for u in range(7):
                        hu_ps = hps.tile([P, NCH], dt.float32, tag="h", name=f"h{u}")
                        for k in range(KO):
                            nc.tensor.matmul(
                                hu_ps[:, :nch],
                                w1sb[:, k, P * u : P * u + P],
                                xsb[:, k, :nch],
                                start=(k == 0),
                                stop=(k == KO - 1),
                            )
                        stmp = spool.tile([64, NCH], dt.bfloat16, tag="stmp", name="stmp")
                        if act_mode == "silu":
                            nc.scalar.activation(
                                stmp[:, :nch], hu_ps[0:64, :nch], silu
                            )
                        else:  # silu(g) = g * sigmoid(g); CoreSim lacks Silu
                            nc.scalar.activation(
                                stmp[:, :nch], hu_ps[0:64, :nch], sigmoid
                            )
                            nc.vector.tensor_mul(
                                stmp[:, :nch], stmp[:, :nch], hu_ps[0:64, :nch]
                            )
                        lo = 64 * (u % 2)
                        nc.vector.tensor_mul(
                            a_tiles[u // 2][lo : lo + 64, :nch],
                            stmp[:, :nch],
                            hu_ps[64:128, :nch],
                        )

                    # gemm2 (emitted one chunk behind gemm1 so the PE never
                    # waits on this chunk's silu/mul chain). Flipped layout:
                    # for each 128-col H tile m: psum[128, nch] accumulates
                    # over 4 ffn k-tiles (last only 64 partitions), cast to
                    # osb[:, m, :nch], then store the chunk in two packed
                    # halves on the sync ring.
                    def gemm2(nch=nch, r=r, a_tiles=a_tiles, w2sb=w2sb):
                        osb = opool.tile(
                            [P, KO2, NCH], dt.bfloat16, tag="osb", name="osb"
                        )
                        for m in range(KO2):
                            o_ps = ops.tile([P, NCH], dt.float32, tag="o", name=f"o{m}")
                            for k in range(K2):
                                kp = P if k < K2 - 1 else 64
                                nc.tensor.matmul(
                                    o_ps[:, :nch],
                                    w2sb[:kp, k, P * m : P * m + P],
                                    a_tiles[k][:kp, :nch],
                                    start=(k == 0),
                                    stop=(k == K2 - 1),
                                )
                            if m % 2 == 0:
                                nc.vector.tensor_copy(
                                    osb[:, m, :nch], o_ps[:, :nch]
                                )
                            else:
                                nc.scalar.activation(
                                    osb[:, m, :nch], o_ps[:, :nch], copyf
                                )
                            if m == KO2 // 2 - 1 or m == KO2 - 1:
                                lo = 0 if m < KO2 // 2 else KO2 // 2
                                off = KO2 * r + lo * nch
                                nc.sync.dma_start(
                                    outp[:, off : off + (KO2 // 2) * nch],
                                    osb[:, lo : lo + KO2 // 2, :nch],
                                )

                    if pending_gemm2 is not None:
                        pending_gemm2()
                    pending_gemm2 = gemm2
                    chunk_idx += 1
            pending_gemm2()

    nc.compile()
    return nc


def _prepare_inputs(hidden_states, w1, w2, chunks):
    """Host-side shard/layout/cast. Returns (xTp, [w1c], [w2m], [w2t])."""
    x = np.asarray(hidden_states, dtype=np.float32)
    w1 = np.asarray(w1, dtype=np.float32)
    w2 = np.asarray(w2, dtype=np.float32)

    xb = x.astype(BF16)          # [R, H]
    w1b = w1.astype(BF16)        # [E, H, 2F]
    w2b = w2.astype(BF16)        # [E, F, H]

    # xTflat[p, k, r] = x[r, 128*k + p]
    xTflat = np.ascontiguousarray(xb.T.reshape(KO, P, R).transpose(1, 0, 2))
    # per-chunk packed: chunk at row r, size nch -> cols [KO*r : KO*(r+nch)]
    xTp = np.empty((P, KO * R), dtype=BF16)
    for (_, r, nch) in chunks:
        xTp[:, KO * r : KO * (r + nch)] = xTflat[:, :, r : r + nch].reshape(
            P, KO * nch
        )

    w1cs, w2ms, w2ts = [], [], []
    for c in range(8):
        gate = w1b[:, :, c * FC : (c + 1) * FC]
        up = w1b[:, :, F + c * FC : F + (c + 1) * FC]
        # interleave 64-channel blocks: [G0|U0|G1|U1|...|G6|U6] so each
        # 128-column m-slice u packs gate_u in psum partitions 0:64 and
        # up_u in 64:128.
        w1cat = np.ascontiguousarray(
            np.stack(
                [gate.reshape(E, H, FC // 64, 64), up.reshape(E, H, FC // 64, 64)],
                axis=3,
            ).reshape(E, H, 2 * FC)
        )
        w1c = np.ascontiguousarray(
            w1cat.reshape(E, KO, P, 2 * FC).transpose(0, 2, 1, 3)
        )
        wslice = w2b[:, c * FC : (c + 1) * FC, :]      # [E, 448, H]
        w2m = np.ascontiguousarray(
            wslice[:, : 3 * P, :].reshape(E, 3, P, H).transpose(0, 2, 1, 3)
        )                                               # [E, P, 3, H]
        w2t = np.ascontiguousarray(wslice[:, 3 * P :, :])  # [E, 64, H]
        w1cs.append(w1c)
        w2ms.append(w2m)
        w2ts.append(w2t)
    return xTp, w1cs, w2ms, w2ts


def kernel(hidden_states, w1, w2, rows_for_experts):
    global LAST_RESULT
    from concourse.bass_utils import run_bass_kernel_spmd

    segs = _segments(np.asarray(rows_for_experts))
    if not segs:
        return np.zeros((R, H), dtype=np.float32)
    key = tuple(segs)
    nc = _PROGRAM_CACHE.get(key)
    if nc is None:
        nc = _build_program(segs)
        _PROGRAM_CACHE[key] = nc

    chunks = _chunk_list(segs)
    xTp, w1cs, w2ms, w2ts = _prepare_inputs(hidden_states, w1, w2, chunks)
    in_maps = [
        {"xTp": xTp, "w1c": w1cs[c], "w2m": w2ms[c], "w2t": w2ts[c]}
        for c in range(8)
    ]
    res = run_bass_kernel_spmd(nc, in_maps, core_ids=list(range(8)))
    LAST_RESULT = res

    acc = np.zeros((R, H), dtype=np.float32)
    for c in range(8):
        flat = res.results[c]["outp"]  # [P, KO2*R] bf16, per-chunk packed
        for (_, r, nch) in chunks:
            blk = flat[:, KO2 * r : KO2 * (r + nch)].reshape(P, KO2, nch)
            # out[r+j, m*128+p] = blk[p, m, j]
            acc[r : r + nch] += (
                blk.transpose(1, 0, 2).reshape(H, nch).T.astype(np.float32)
            )
    return acc


# revision 12
# speedup vs baseline: 1.1043x; 1.1043x over previous
"""Trainium2 Bass kernel for MixtralBlockSparseTop2MLP grouped-GEMM MoE.

Problem: 4096 rows (sorted by expert), 8 experts, hidden=1024, ffn=3584.
  out[r] = silu(x[r] @ W1g[e(r)]) * (x[r] @ W1u[e(r)]) @ W2[e(r)]

Sharding: tensor-parallel over the ffn dimension. Each of the 8 cores gets
a 448-channel slice of every expert's gate/up/down weights and computes a
partial output for ALL 4096 rows; the host sums the 8 partials. All cores
run the identical program (segment structure baked from rows_for_experts at
call time), so one SPMD NEFF serves all 8 cores with per-core weight data.

v2 design notes (vs the 203us baseline):
  - gemm2 is flipped: stationary = w2 [ffn_k x 128 H-cols], moving = the
    a-tiles [ffn_k x nch rows], psum = [128 H-cols x nch]. Cost scales with
    actual chunk rows instead of ceil(rows/128) full 512-col slices, which
    removes the partial-slice and zero-pad PE waste entirely. Output is
    H-major ([P, KO2, rows]); the host transposes while summing partials.
  - the 4th gemm2 k-tile is zero-padded to 128 partitions (448 = 3.5
    k-tiles): measured HW penalty of ~100ns per tile_size 128<->64 switch
    makes 64-partition matmuls a net loss; zeros come from host-padded w2
    and a persistent pair of zeroed a3 tiles (only rows 0:64 rewritten).
  - chunks are balanced per segment (n split into ceil(n/512) near-equal
    parts) so no chunk sits at the ~100ns/instruction issue floor with only
    8-15 rows of work.
  - x and out use per-chunk packed DRAM layouts (one contiguous line per
    partition per transfer) - no padded-row DMA waste, minimal descriptors.
  - DMA rings: gpsimd(SWDGE)=x loads only, sync(HWDGE)=w1 + output stores,
    scalar(HWDGE)=w2. psum->sbuf casts alternate vector/scalar.

Compute dtype: bf16 matmul inputs with fp32 PSUM accumulation (fp32 matmul
is 4x slower; fp8 fails the 2e-2 gate - measured 3.8-6.5% rel err).
"""

import sys

sys.path.insert(0, "/opt/trn_rl_repo")

import numpy as np
import ml_dtypes

E, R, H, F = 8, 1024 * 4, 1024, 3584
FC = F // 8          # 448 ffn channels per core
P = 128
KO = H // P          # 8 k-tiles for gemm1
K2 = (FC + P - 1) // P   # 4 k-tiles for gemm2 (last has 64 rows)
KO2 = H // P         # 8 output m-tiles for gemm2
NCH = 512            # max row-chunk (PSUM bank free dim)

BF16 = ml_dtypes.bfloat16

# test.py introspection: last BassKernelResults from run_bass_kernel_spmd
LAST_RESULT = None

_PROGRAM_CACHE = {}


def _segments(rows_for_experts):
    """[(expert, row_start, n_rows)] for experts with n_rows > 0."""
    segs = []
    r0 = 0
    for e in range(E):
        n = int(rows_for_experts[e])
        if n > 0:
            segs.append((e, r0, n))
        r0 += n
    # largest segment first (amortizes the startup weight-load stall),
    # smallest last (short end-of-kernel gemm2+store chain).
    segs.sort(key=lambda s: -s[2])
    return segs


def _chunk_sizes(n):
    """Split n rows into ceil(n/NCH) near-equal chunks (balanced so no
    chunk is tiny enough to be instruction-issue bound)."""
    k = (n + NCH - 1) // NCH
    base, rem = divmod(n, k)
    return [base + 1] * rem + [base] * (k - rem)


def _chunk_list(segments):
    """[(expert, row_start, nch)] in program iteration order."""
    out = []
    for (e, r0, n_e) in segments:
        c0 = 0
        for nch in _chunk_sizes(n_e):
            out.append((e, r0 + c0, nch))
            c0 += nch
    return out


def _build_program(segments, act_mode="silu"):
    import concourse.mybir as mybir
    import concourse.tile as tile
    from concourse import bacc

    dt = mybir.dt
    nc = bacc.Bacc(None, target_bir_lowering=False, debug=False)

    chunks = _chunk_list(segments)
    n_chunks = len(chunks)
    # per-chunk packed x: chunk c occupies columns [KO*r : KO*(r+nch)] as a
    # row-major [KO, nch] block per partition (one contiguous line each).
    xTp = nc.declare_dram_parameter("xTp", [P, KO * R], dt.bfloat16, isOutput=False)
    w1 = nc.declare_dram_parameter("w1c", [E, P, KO, 2 * FC], dt.bfloat16, isOutput=False)
    w2 = nc.declare_dram_parameter("w2c", [E, P, K2, H], dt.bfloat16, isOutput=False)
    # per-chunk packed out: chunk c occupies columns [KO2*r : KO2*(r+nch)]
    # as a [KO2, nch] block per partition; out[r, m*128+p] = outp[p, ...].
    outp = nc.declare_dram_parameter("outp", [P, KO2 * R], dt.bfloat16, isOutput=True)

    silu = mybir.ActivationFunctionType.Silu
    sigmoid = mybir.ActivationFunctionType.Sigmoid
    copyf = mybir.ActivationFunctionType.Copy

    with tile.TileContext(nc) as tc:
        with (
            tc.tile_pool(name="w1p", bufs=4) as w1p,
            tc.tile_pool(name="w2p", bufs=3) as w2p,
            tc.tile_pool(name="xp", bufs=4) as xp,
            tc.tile_pool(name="apool", bufs=2) as apool,
            tc.tile_pool(name="a3pool", bufs=1) as a3pool,
            tc.tile_pool(name="spool", bufs=2) as spool,
            tc.tile_pool(name="opool", bufs=3) as opool,
            tc.tile_pool(name="hps", bufs=4, space="PSUM") as hps,
            tc.tile_pool(name="ops", bufs=4, space="PSUM") as ops,
        ):
            # a3 holds only channels 384:448 in rows 0:64; rows 64:128 must
            # be exact zeros (they multiply the zero-padded w2 k3 rows).
            # Persistent ping-pong pair, zeroed once.
            a3_tiles = [
                a3pool.tile([P, NCH], dt.bfloat16, tag=f"a3_{i}", name=f"a3_{i}")
                for i in range(2)
            ]
            for t3 in a3_tiles:
                nc.vector.memset(t3[:], 0.0)

            chunk_idx = 0
            first = True
            pending_gemm2 = None
            for (seg_idx, (e, r0, n_e)) in enumerate(segments):
                w1sb = w1p.tile([P, KO, 2 * FC], dt.bfloat16, tag="w1sb")
                if first:
                    # prologue: interleave x-chunk0 / w1 k-slices so both
                    # k0 pieces are first on their rings; split w1 k0 so
                    # the very first matmul's 128 columns land fastest.
                    nch0 = _chunk_sizes(n_e)[0]
                    xsb0 = xp.tile([P, KO, NCH], dt.bfloat16, tag="xsb")
                    for k in range(KO):
                        off = KO * r0 + k * nch0
                        nc.gpsimd.dma_start(
                            xsb0[:, k, :nch0], xTp[:, off : off + nch0]
                        )
                        if k == 0:
                            nc.sync.dma_start(w1sb[:, 0, 0:P], w1[e, :, 0, 0:P])
                            nc.sync.dma_start(
                                w1sb[:, 0, P:], w1[e, :, 0, P:]
                            )
                        else:
                            nc.sync.dma_start(w1sb[:, k, :], w1[e, :, k, :])
                    first = False
                elif seg_idx < 3:
                    # early segments: per-k slices so gemm1 can start on k0
                    # while later slices are still in flight (startup is
                    # HBM-bound; a whole-expert DMA lands all-or-nothing).
                    xsb0 = None
                    for k in range(KO):
                        nc.sync.dma_start(w1sb[:, k, :], w1[e, :, k, :])
                else:
                    xsb0 = None
                    nc.sync.dma_start(w1sb[:], w1[e])
                w2sb = w2p.tile([P, K2, H], dt.bfloat16, tag="w2sb")
                nc.scalar.dma_start(w2sb[:], w2[e])

                c0 = 0
                for nch in _chunk_sizes(n_e):
                    r = r0 + c0
                    c0 += nch

                    if xsb0 is not None:
                        xsb, xsb0 = xsb0, None
                    else:
                        xsb = xp.tile([P, KO, NCH], dt.bfloat16, tag="xsb")
                        off = KO * r
                        nc.gpsimd.dma_start(
                            xsb[:, :, :nch], xTp[:, off : off + KO * nch]
                        )

                    # gemm1: 7 packed m-slices [gate_u(64) | up_u(64)];
                    # psum_u partitions 0:64 = gate, 64:128 = up.
                    # silu via ACT into a 64-row tmp, then DVE cross-base
                    # multiply into the packed a k-tiles. a3 rows 64:128
                    # are never written or read (448 = 3.5 k-tiles).
                    a_tiles = [
                        apool.tile([P, NCH], dt.bfloat16, tag=f"a{j}", name=f"a{j}")
                        if j < 3
                        else a3_tiles[chunk_idx % 2]
                        for j in range(K2)
                    ]
                    for u in range(7):
                        hu_ps = hps.tile([P, NCH], dt.float32, tag="h", name=f"h{u}")
                        for k in range(KO):
                            nc.tensor.matmul(
                                hu_ps[:, :nch],
                                w1sb[:, k, P * u : P * u + P],
                                xsb[:, k, :nch],
                                start=(k == 0),
                                stop=(k == KO - 1),
                            )
                        stmp = spool.tile([64, NCH], dt.bfloat16, tag="stmp", name="stmp")
                        if act_mode == "silu":
                            nc.scalar.activation(
                                stmp[:, :nch], hu_ps[0:64, :nch], silu
                            )
                        else:  # silu(g) = g * sigmoid(g); CoreSim lacks Silu
                            nc.scalar.activation(
                                stmp[:, :nch], hu_ps[0:64, :nch], sigmoid
                            )
                            nc.vector.tensor_mul(
                                stmp[:, :nch], stmp[:, :nch], hu_ps[0:64, :nch]
                            )
                        lo = 64 * (u % 2)
                        nc.vector.tensor_mul(
                            a_tiles[u // 2][lo : lo + 64, :nch],
                            stmp[:, :nch],
                            hu_ps[64:128, :nch],
                        )

                    # gemm2 (emitted one chunk behind gemm1 so the PE never
                    # waits on this chunk's silu/mul chain). Flipped layout:
                    # for each 128-col H tile m: psum[128, nch] accumulates
                    # over 4 ffn k-tiles (last only 64 partitions), cast to
                    # osb[:, m, :nch], then store the chunk in two packed
                    # halves on the sync ring.
                    is_last = chunk_idx == n_chunks - 1
                    def gemm2(nch=nch, r=r, a_tiles=a_tiles, w2sb=w2sb,
                              store_every=(2 if is_last else 4)):
                        osb = opool.tile(
                            [P, KO2, NCH], dt.bfloat16, tag="osb", name="osb"
                        )
                        for m in range(KO2):
                            o_ps = ops.tile([P, NCH], dt.float32, tag="o", name=f"o{m}")
                            for k in range(K2):
                                nc.tensor.matmul(
                                    o_ps[:, :nch],
                                    w2sb[:, k, P * m : P * m + P],
                                    a_tiles[k][:, :nch],
                                    start=(k == 0),
                                    stop=(k == K2 - 1),
                                )
                            if m % 2 == 0:
                                nc.vector.tensor_copy(
                                    osb[:, m, :nch], o_ps[:, :nch]
                                )
                            else:
                                nc.scalar.activation(
                                    osb[:, m, :nch], o_ps[:, :nch], copyf
                                )
                            if (m + 1) % store_every == 0:
                                lo = m + 1 - store_every
                                off = KO2 * r + lo * nch
                                nc.sync.dma_start(
                                    outp[:, off : off + store_every * nch],
                                    osb[:, lo : m + 1, :nch],
                                )

                    if pending_gemm2 is not None:
                        pending_gemm2()
                    pending_gemm2 = gemm2
                    chunk_idx += 1
            pending_gemm2()

    nc.compile()
    return nc


def _prepare_inputs(hidden_states, w1, w2, chunks):
    """Host-side shard/layout/cast. Returns (xTp, [w1c], [w2c])."""
    x = np.asarray(hidden_states, dtype=np.float32)
    w1 = np.asarray(w1, dtype=np.float32)
    w2 = np.asarray(w2, dtype=np.float32)

    xb = x.astype(BF16)          # [R, H]
    w1b = w1.astype(BF16)        # [E, H, 2F]
    w2b = w2.astype(BF16)        # [E, F, H]

    # xTflat[p, k, r] = x[r, 128*k + p]
    xTflat = np.ascontiguousarray(xb.T.reshape(KO, P, R).transpose(1, 0, 2))
    # per-chunk packed: chunk at row r, size nch -> cols [KO*r : KO*(r+nch)]
    xTp = np.empty((P, KO * R), dtype=BF16)
    for (_, r, nch) in chunks:
        xTp[:, KO * r : KO * (r + nch)] = xTflat[:, :, r : r + nch].reshape(
            P, KO * nch
        )

    w1cs, w2cs = [], []
    for c in range(8):
        gate = w1b[:, :, c * FC : (c + 1) * FC]
        up = w1b[:, :, F + c * FC : F + (c + 1) * FC]
        # interleave 64-channel blocks: [G0|U0|G1|U1|...|G6|U6] so each
        # 128-column m-slice u packs gate_u in psum partitions 0:64 and
        # up_u in 64:128.
        w1cat = np.ascontiguousarray(
            np.stack(
                [gate.reshape(E, H, FC // 64, 64), up.reshape(E, H, FC // 64, 64)],
                axis=3,
            ).reshape(E, H, 2 * FC)
        )
        w1c = np.ascontiguousarray(
            w1cat.reshape(E, KO, P, 2 * FC).transpose(0, 2, 1, 3)
        )
        w2pad = np.zeros((E, K2 * P, H), dtype=BF16)
        w2pad[:, :FC, :] = w2b[:, c * FC : (c + 1) * FC, :]
        w2c = np.ascontiguousarray(
            w2pad.reshape(E, K2, P, H).transpose(0, 2, 1, 3)
        )                                               # [E, P, K2, H]
        w1cs.append(w1c)
        w2cs.append(w2c)
    return xTp, w1cs, w2cs


def kernel(hidden_states, w1, w2, rows_for_experts):
    global LAST_RESULT
    from concourse.bass_utils import run_bass_kernel_spmd

    segs = _segments(np.asarray(rows_for_experts))
    if not segs:
        return np.zeros((R, H), dtype=np.float32)
    key = tuple(segs)
    nc = _PROGRAM_CACHE.get(key)
    if nc is None:
        nc = _build_program(segs)
        _PROGRAM_CACHE[key] = nc

    chunks = _chunk_list(segs)
    xTp, w1cs, w2cs = _prepare_inputs(hidden_states, w1, w2, chunks)
    in_maps = [
        {"xTp": xTp, "w1c": w1cs[c], "w2c": w2cs[c]} for c in range(8)
    ]
    res = run_bass_kernel_spmd(nc, in_maps, core_ids=list(range(8)))
    LAST_RESULT = res

    acc = np.zeros((R, H), dtype=np.float32)
    for c in range(8):
        flat = res.results[c]["outp"]  # [P, KO2*R] bf16, per-chunk packed
        for (_, r, nch) in chunks:
            blk = flat[:, KO2 * r : KO2 * (r + nch)].reshape(P, KO2, nch)
            # out[r+j, m*128+p] = blk[p, m, j]
            acc[r : r + nch] += (
                blk.transpose(1, 0, 2).reshape(H, nch).T.astype(np.float32)
            )
    return acc
